# revision 25
# baseline (speedup 1.0000x reference)
"""Trainium2 Bass kernel for nn_BPF_Attention (B=4, N=2048, C=768, H=12).

Sharding: 8 cores = 4 batches x 2 head-groups (6 heads each).
Per core, for its (batch b, head-group g):
  qT/kT = (w_qkv_g^T x_b^T) in [d, n] layout (no on-device transpose)
  v     = x_b @ w_v_g in natural [n, d] layout (+ ones column for denom)
  S^T[k,q] = kT-chunks x qT  (PE, bf16, row-tiled 64-contraction pairs)
  P^T = exp(0.125*S^T) (ACT) * m01T (DVE)
  outT[d,q] + denom row = v_ones^T x P^T (PE, accumulated over k-tiles)
  attn = outT * recip(denom) broadcast; y = attn^T-chunks @ w_proj_g

Structure: flat software pipeline over all (qc, jp, kt) slots; the ACT
exp stream is the critical path.  v for k-tiles 2..15, qkv projections
for head-pairs 1,2, and the output projection of each q-chunk are
drained one group per slot as filler PE work; the prologue (qk of
pair0 + v0,v1) and the tail projection evacuate PSUM via the
otherwise-idle Scalar engine.  Normalization runs per (qc, head-pair):
denom rows -> DRAM, reciprocal, bf16 broadcast back, fused into the
attn tile before projection.  y partials are written bf16; host sums
the two head-group partials per batch in f32 and adds b_proj.
"""

import numpy as np
import ml_dtypes

import concourse.bass as bass
import concourse.tile as tile
import concourse.bacc as bacc
import concourse.mybir as mybir
from concourse.bass_utils import run_bass_kernel_spmd

BF16 = ml_dtypes.bfloat16
F32 = mybir.dt.float32
BF = mybir.dt.bfloat16

B, N, C = 4, 2048, 768
H, D = 12, 64
HG = 6                      # heads per core
SCALE = D ** -0.5           # 0.125
NCORES = 8
WQ = 512                    # query-chunk width
NKT = N // 128              # 16 k-tiles
NQC = N // WQ               # 4 q-chunks
EXP = mybir.ActivationFunctionType.Exp


def _build_nc(loop=0):
    nc = bacc.Bacc(
        "TRN2",
        target_bir_lowering=False,
        debug=False,
        enable_asserts=True,
        num_devices=NCORES,
    )
    xT_d = nc.dram_tensor("xT", [6, 128, N], BF, kind="ExternalInput")
    w_d = nc.dram_tensor("wqkv", [6, 128, 1152], BF, kind="ExternalInput")
    m_d = nc.dram_tensor("maskT", [NKT, 128, N], BF, kind="ExternalInput")
    wp_d = nc.dram_tensor("wp", [3, 128, C], BF, kind="ExternalInput")
    y_d = nc.dram_tensor("y", [N, C], BF, kind="ExternalOutput")

    with tile.TileContext(nc) as tc:
        if loop:
            with tc.For_i(0, loop, 1):
                _kernel_body(tc, xT_d, w_d, m_d, wp_d, y_d)
        else:
            _kernel_body(tc, xT_d, w_d, m_d, wp_d, y_d)
    nc.compile()
    return nc


def _kernel_body(tc, xT_d, w_d, m_d, wp_d, y_d):
    nc = tc.nc
    from contextlib import ExitStack

    with ExitStack() as ctx:
        persist = ctx.enter_context(tc.tile_pool(name="persist", bufs=1))
        work = ctx.enter_context(tc.tile_pool(name="work", bufs=6))
        work2 = ctx.enter_context(tc.tile_pool(name="work2", bufs=4))

        # ---- persistent SBUF tensors ----
        xT_sb = persist.tile([128, 6, N], BF, tag="xT")
        w_sb = persist.tile([128, 6, 1152], BF, tag="w")
        qT_sb = persist.tile([128, 3, N], BF, tag="qT")
        kT_sb = persist.tile([128, 3, N], BF, tag="kT")
        v_ones = persist.tile([128, NKT, HG, 65], BF, tag="vo")
        m01_sb = persist.tile([128, NKT, N], BF, tag="m01")
        attn_sb = persist.tile([128, 3, N], BF, tag="attn")
        wp_sb = persist.tile([128, 3, C], BF, tag="wp")

        for cc in range(6):
            nc.sync.dma_start(out=w_sb[:, cc, :], in_=w_d[cc])
        for cc in range(6):
            nc.sync.dma_start(out=xT_sb[:, cc, :], in_=xT_d[cc])
        for kt in range(NKT):
            nc.gpsimd.dma_start(out=m01_sb[:, kt, :], in_=m_d[kt])
        for j in range(3):
            nc.gpsimd.dma_start(out=wp_sb[:, j, :], in_=wp_d[j])

        nc.vector.memset(v_ones[:, :, :, 64:65], 1.0)

        dram = ctx.enter_context(
            tc.tile_pool(name="dscratch", bufs=1, space="DRAM")
        )
        rscratch = dram.tile([NQC * HG, WQ], F32, tag="rs")
        rscratchb = dram.tile([NQC * HG, WQ], BF, tag="rsb")

        with tc.tile_pool(name="ps_s", bufs=2, space="PSUM") as ps_s, \
             tc.tile_pool(name="ps_o", bufs=3, space="PSUM") as ps_o, \
             tc.tile_pool(name="ps_q", bufs=1, space="PSUM") as ps_q:

            # ---------- emitters ----------
            def emit_qk_group(cp, g, pool, on_act):
                dest = qT_sb if cp < 3 else kT_sb
                j = cp % 3
                ps = pool.tile([128, WQ], F32, tag="q", name="ps_qk")
                for cc in range(6):
                    nc.tensor.matmul(
                        ps[:, 0:WQ],
                        w_sb[:, cc, cp * 128:(cp + 1) * 128],
                        xT_sb[:, cc, g * WQ:(g + 1) * WQ],
                        start=(cc == 0),
                        stop=(cc == 5),
                    )
                dst = dest[:, j, g * WQ:(g + 1) * WQ]
                if on_act:
                    nc.scalar.copy(out=dst, in_=ps[:, 0:WQ])
                else:
                    nc.vector.tensor_copy(out=dst, in_=ps[:, 0:WQ])

            def emit_v(nt, pool, on_act):
                ps = pool.tile([128, WQ], F32, tag="q", name="ps_v")
                for cc in range(6):
                    nc.tensor.matmul(
                        ps[:, 0:384],
                        xT_sb[:, cc, nt * 128:(nt + 1) * 128],
                        w_sb[:, cc, 768:1152],
                        start=(cc == 0),
                        stop=(cc == 5),
                    )
                dst = v_ones[:, nt, :, 0:64]
                src = ps[:, 0:384].rearrange("p (h d) -> p h d", h=HG)
                if on_act:
                    nc.scalar.copy(out=dst, in_=src)
                else:
                    nc.vector.tensor_copy(out=dst, in_=src)

            def emit_norm_jp(qc, jp):
                q0 = qc * WQ
                r0 = qc * HG + 2 * jp
                pair = work2.tile([2, WQ], F32, tag="pair")
                nc.sync.dma_start(out=pair[:, :], in_=rscratch[r0:r0 + 2, :])
                nc.vector.reciprocal_approx_fast(out=pair[:, :], in_=pair[:, :])
                pairb = work2.tile([2, WQ], BF, tag="pairb")
                nc.vector.tensor_copy(out=pairb[:, :], in_=pair[:, :])
                nc.sync.dma_start(
                    out=rscratchb[r0:r0 + 2, :], in_=pairb[:, :]
                )
                bc = work2.tile([128, WQ], BF, tag="bcj")
                for half in range(2):
                    row = rscratchb[r0 + half:r0 + half + 1, :]
                    bsrc = bass.AP(
                        tensor=row.tensor,
                        offset=row.offset,
                        ap=[[0, 64], [1, WQ]],
                    )
                    nc.sync.dma_start(
                        out=bc[64 * half:64 * half + 64, :], in_=bsrc
                    )
                nc.vector.tensor_mul(
                    attn_sb[:, jp, q0:q0 + WQ],
                    attn_sb[:, jp, q0:q0 + WQ],
                    bc[:, :],
                )

            def emit_proj(nt, pool, on_act):
                for colh in range(2):
                    c0 = colh * 384
                    psy = pool.tile([128, WQ], F32, tag="q", name="ps_y")
                    for j in range(3):
                        nc.tensor.matmul(
                            psy[:, 0:384],
                            attn_sb[:, j, nt * 128:(nt + 1) * 128],
                            wp_sb[:, j, c0:c0 + 384],
                            start=(j == 0),
                            stop=(j == 2),
                        )
                    ysb = work2.tile([128, 384], BF, tag="ysb")
                    if on_act:
                        nc.scalar.copy(out=ysb[:, :], in_=psy[:, 0:384])
                    else:
                        nc.vector.tensor_copy(out=ysb[:, :], in_=psy[:, 0:384])
                    nc.gpsimd.dma_start(
                        out=y_d[nt * 128:(nt + 1) * 128, c0:c0 + 384],
                        in_=ysb[:, :],
                    )

            # ---------- prologue: qk(jp0) + v0,v1 on idle ACT ----------
            for cp in (0, 3):
                for g in range(4):
                    emit_qk_group(cp, g, ps_q, on_act=True)
            for nt in range(2):
                emit_v(nt, ps_q, on_act=True)

            # ---------- filler schedule: slot -> [callables] ----------
            # Constraints: v(nt) must be emitted at slot <= nt (consumer
            # O(nt) is emitted at slot nt+1); q/k plane groups before the
            # emit_s that reads them (jp1 from slot 16, jp2 from slot 32,
            # k-group g feeds k-tiles 4g..4g+3).
            fillers = {}

            def add_filler(slot, fn):
                fillers.setdefault(slot, []).append(fn)

            def f_v(nt):
                return lambda: emit_v(nt, ps_q, False)

            def f_qk(cp, g):
                return lambda: emit_qk_group(cp, g, ps_q, False)

            add_filler(0, f_v(2))
            for idx, nt in enumerate(range(3, NKT)):
                add_filler(1 + idx, f_v(nt))            # slots 1..13
            # jp1 planes needed from slot 64 (q g_qc at 64+16qc, k g_j at
            # 64+4j), jp2 from slot 128; spread groups to avoid overloading
            # the PE while v fillers drain.
            qk_sched = [
                (30, 4, 0), (34, 1, 0), (38, 4, 1), (44, 4, 2), (50, 4, 3),
                (56, 1, 1), (70, 1, 2), (86, 1, 3),
                (94, 5, 0), (98, 2, 0), (102, 5, 1), (108, 5, 2),
                (114, 5, 3), (120, 2, 1), (134, 2, 2), (150, 2, 3),
            ]
            for slot, cp, g in qk_sched:
                add_filler(slot, f_qk(cp, g))
            # proj(qc) after norm of (jp2, qc), which lands at slot
            # 128 + 16*(qc+1); spread the 4 n-tiles 3 slots apart
            for qc in range(NQC - 1):
                base = 128 + 16 * (qc + 1)
                for i in range(4):
                    add_filler(
                        base + 2 + 3 * i,
                        (lambda t: lambda: emit_proj(t, ps_q, False))(qc * 4 + i),
                    )

            # ---------- flat attention pipeline ----------
            # jp-outer: head-pair jp sweeps all 4 q-chunks before the next
            # pair starts, so jp1/jp2 qk planes are not needed until slots
            # 64/128 and the single-bank filler queue has ample slack.
            slots = [
                (qc, jp, kt)
                for jp in range(3)
                for qc in range(NQC)
                for kt in range(NKT)
            ]
            pT_pend = {}
            po_pend = {}

            def emit_s(qc, jp, kt):
                q0 = qc * WQ
                k0 = kt * 128
                psum_s = ps_s.tile([128, 2 * WQ], F32, tag="s")
                nc.tensor.matmul(
                    psum_s[:, 0:WQ],
                    kT_sb[0:64, jp, k0:k0 + 128],
                    qT_sb[0:64, jp, q0:q0 + WQ],
                    start=True,
                    stop=True,
                )
                nc.tensor.matmul(
                    psum_s[:, WQ:2 * WQ],
                    kT_sb[64:128, jp, k0:k0 + 128],
                    qT_sb[64:128, jp, q0:q0 + WQ],
                    start=True,
                    stop=True,
                )
                praw = work.tile([128, 2 * WQ], BF, tag="praw")
                nc.scalar.activation(
                    out=praw[:, :], in_=psum_s[:, :], func=EXP, scale=SCALE
                )
                pT = work.tile([128, 2 * WQ], BF, tag="pt")
                msl = m01_sb[:, kt, q0:q0 + WQ]
                msrc = bass.AP(
                    tensor=msl.tensor,
                    offset=msl.offset,
                    ap=[list(msl.ap[0]), [0, 2], [1, WQ]],
                )
                nc.vector.tensor_mul(
                    pT.rearrange("p (a b) -> p a b", a=2),
                    praw.rearrange("p (a b) -> p a b", a=2),
                    msrc,
                )
                pT_pend[(qc, jp, kt)] = pT

            def emit_o(qc, jp, kt):
                q0 = qc * WQ
                if kt == 0:
                    po_pend[(qc, jp)] = (
                        ps_o.tile([65, WQ], F32, tag="o", name="psum_oe"),
                        ps_o.tile([65, WQ], F32, tag="o", name="psum_oo"),
                    )
                psum_oe, psum_oo = po_pend[(qc, jp)]
                pT = pT_pend.pop((qc, jp, kt))
                nc.tensor.matmul(
                    psum_oe[:, :],
                    v_ones[:, kt, 2 * jp, :],
                    pT[:, 0:WQ],
                    start=(kt == 0),
                    stop=(kt == NKT - 1),
                )
                nc.tensor.matmul(
                    psum_oo[:, :],
                    v_ones[:, kt, 2 * jp + 1, :],
                    pT[:, WQ:2 * WQ],
                    start=(kt == 0),
                    stop=(kt == NKT - 1),
                )
                if kt == NKT - 1:
                    for half, po in ((0, psum_oe), (1, psum_oo)):
                        off = 64 * half
                        nc.vector.tensor_copy(
                            out=attn_sb[off:off + 64, jp, q0:q0 + WQ],
                            in_=po[0:64, :],
                        )
                        den = work2.tile([1, WQ], F32, tag="dn")
                        nc.vector.tensor_copy(out=den[:, :], in_=po[64:65, :])
                        row = qc * HG + 2 * jp + half
                        nc.sync.dma_start(
                            out=rscratch[row:row + 1, :], in_=den[:, :]
                        )
                    del po_pend[(qc, jp)]
                    emit_norm_jp(qc, jp)

            emit_s(*slots[0])
            for fn in fillers.get(0, ()):
                fn()
            for i in range(1, len(slots)):
                emit_s(*slots[i])
                emit_o(*slots[i - 1])
                for fn in fillers.get(i, ()):
                    fn()
            emit_o(*slots[-1])

            # ---------- tail: last q-chunk projection ----------
            for nt in range((NQC - 1) * 4, NQC * 4):
                emit_proj(nt, ps_q, on_act=True)


def _prep_inputs(x, mask, w_qkv, w_proj):
    """Build the 8 per-core input maps."""
    x = np.asarray(x, dtype=np.float32)
    mask = np.asarray(mask)
    w_qkv = np.asarray(w_qkv, dtype=np.float32)
    w_proj = np.asarray(w_proj, dtype=np.float32)

    m01T = np.ascontiguousarray((~mask).T.astype(np.float32)).astype(BF16)
    m01T = m01T.reshape(NKT, 128, N)

    w3 = w_qkv.reshape(C, 3, H, D)
    wp3 = w_proj.reshape(H, D, C)

    in_maps = []
    for core in range(NCORES):
        b, g = core // 2, core % 2
        hs = slice(g * HG, (g + 1) * HG)
        xT = np.ascontiguousarray(x[b].T).astype(BF16).reshape(6, 128, N)
        wq = w3[:, 0, hs, :].reshape(C, HG * D)
        wk = w3[:, 1, hs, :].reshape(C, HG * D)
        wv = w3[:, 2, hs, :].reshape(C, HG * D)
        wg = np.concatenate([wq, wk, wv], axis=1).astype(BF16)
        wg = np.ascontiguousarray(wg).reshape(6, 128, 1152)
        wp = np.ascontiguousarray(wp3[hs].reshape(3, 128, C)).astype(BF16)
        in_maps.append({"xT": xT, "wqkv": wg, "maskT": m01T, "wp": wp})
    return in_maps


_NC_CACHE = {}


def run_cores(in_maps, trace=False, **kw):
    if "nc" not in _NC_CACHE:
        _NC_CACHE["nc"] = _build_nc()
    nc = _NC_CACHE["nc"]
    return run_bass_kernel_spmd(
        nc, in_maps, core_ids=list(range(NCORES)), trace=trace, **kw
    )


def kernel(x, mask, w_qkv, w_proj, b_proj):
    in_maps = _prep_inputs(x, mask, w_qkv, w_proj)
    res = run_cores(in_maps)
    b_proj = np.asarray(b_proj, dtype=np.float32)
    out = np.empty((B, N, C), dtype=np.float32)
    for b in range(B):
        out[b] = (
            res.results[2 * b]["y"].astype(np.float32)
            + res.results[2 * b + 1]["y"].astype(np.float32)
            + b_proj
        )
    return out


# revision 26
# speedup vs baseline: 1.0632x; 1.0632x over previous
"""Trainium2 Bass kernel for nn_BPF_Attention (B=4, N=2048, C=768, H=12).

Sharding: 8 cores = 4 batches x 2 head-groups (6 heads each).
Per core, for its (batch b, head-group g):
  qT/kT = (w_qkv_g^T x_b^T) in [d, n] layout (no on-device transpose)
  v     = x_b @ w_v_g in natural [n, d] layout (+ ones column for denom)
  S^T[k,q] = kT-chunks x qT  (PE, bf16, row-tiled 64-contraction pairs)
  P^T = exp(0.125*S^T) (ACT) * m01T (DVE)
  outT[d,q] + denom row = v_ones^T x P^T (PE, accumulated over k-tiles)
  attn = outT * recip(denom) broadcast; y = attn^T-chunks @ w_proj_g

Structure: flat software pipeline over all (qc, jp, kt) slots; the ACT
exp stream is the critical path.  v for k-tiles 2..15, qkv projections
for head-pairs 1,2, and the output projection of each q-chunk are
drained one group per slot as filler PE work; the prologue (qk of
pair0 + v0,v1) and the tail projection evacuate PSUM via the
otherwise-idle Scalar engine.  Normalization runs per (qc, head-pair):
denom rows -> DRAM, reciprocal, bf16 broadcast back, fused into the
attn tile before projection.  y partials are written bf16; host sums
the two head-group partials per batch in f32 and adds b_proj.
"""

import numpy as np
import ml_dtypes

import concourse.bass as bass
import concourse.tile as tile
import concourse.bacc as bacc
import concourse.mybir as mybir
from concourse.bass_utils import run_bass_kernel_spmd

BF16 = ml_dtypes.bfloat16
F32 = mybir.dt.float32
BF = mybir.dt.bfloat16

B, N, C = 4, 2048, 768
H, D = 12, 64
HG = 6                      # heads per core
SCALE = D ** -0.5           # 0.125
NCORES = 8
WQ = 512                    # query-chunk width
NKT = N // 128              # 16 k-tiles
NQC = N // WQ               # 4 q-chunks
EXP = mybir.ActivationFunctionType.Exp


def _build_nc(loop=0):
    nc = bacc.Bacc(
        "TRN2",
        target_bir_lowering=False,
        debug=False,
        enable_asserts=True,
        num_devices=NCORES,
    )
    xT_d = nc.dram_tensor("xT", [6, 128, N], BF, kind="ExternalInput")
    w_d = nc.dram_tensor("wqkv", [6, 128, 1152], BF, kind="ExternalInput")
    m_d = nc.dram_tensor("maskT", [NKT, 128, N], BF, kind="ExternalInput")
    wp_d = nc.dram_tensor("wp", [3, 128, C], BF, kind="ExternalInput")
    y_d = nc.dram_tensor("y", [N, C], BF, kind="ExternalOutput")

    with tile.TileContext(nc) as tc:
        if loop:
            with tc.For_i(0, loop, 1):
                _kernel_body(tc, xT_d, w_d, m_d, wp_d, y_d)
        else:
            _kernel_body(tc, xT_d, w_d, m_d, wp_d, y_d)
    nc.compile()
    return nc


def _kernel_body(tc, xT_d, w_d, m_d, wp_d, y_d):
    nc = tc.nc
    from contextlib import ExitStack

    with ExitStack() as ctx:
        persist = ctx.enter_context(tc.tile_pool(name="persist", bufs=1))
        work = ctx.enter_context(tc.tile_pool(name="work", bufs=6))
        work2 = ctx.enter_context(tc.tile_pool(name="work2", bufs=4))

        # ---- persistent SBUF tensors ----
        xT_sb = persist.tile([128, 6, N], BF, tag="xT")
        w_sb = persist.tile([128, 6, 1152], BF, tag="w")
        qT_sb = persist.tile([128, 3, N], BF, tag="qT")
        kT_sb = persist.tile([128, 3, N], BF, tag="kT")
        v_ones = persist.tile([128, NKT, HG, 65], BF, tag="vo")
        m01_sb = persist.tile([128, NKT, N], BF, tag="m01")
        attn_sb = persist.tile([128, 3, N], BF, tag="attn")
        wp_sb = persist.tile([128, 3, C], BF, tag="wp")

        for cc in range(6):
            nc.sync.dma_start(out=xT_sb[:, cc, :], in_=xT_d[cc])
            nc.sync.dma_start(out=w_sb[:, cc, :], in_=w_d[cc])
        for kt in range(2):
            nc.sync.dma_start(out=m01_sb[:, kt, :], in_=m_d[kt])
        for j in range(3):
            nc.sync.dma_start(out=wp_sb[:, j, :], in_=wp_d[j])
        for kt in range(2, NKT):
            nc.sync.dma_start(out=m01_sb[:, kt, :], in_=m_d[kt])

        nc.vector.memset(v_ones[:, :, :, 64:65], 1.0)

        dram = ctx.enter_context(
            tc.tile_pool(name="dscratch", bufs=1, space="DRAM")
        )
        rscratch = dram.tile([NQC * HG, WQ], F32, tag="rs")
        rscratchb = dram.tile([NQC * HG, WQ], BF, tag="rsb")

        with tc.tile_pool(name="ps_s", bufs=2, space="PSUM") as ps_s, \
             tc.tile_pool(name="ps_o", bufs=3, space="PSUM") as ps_o, \
             tc.tile_pool(name="ps_q", bufs=1, space="PSUM") as ps_q:

            # ---------- emitters ----------
            def emit_qk_group(cp, g, pool, on_act):
                dest = qT_sb if cp < 3 else kT_sb
                j = cp % 3
                ps = pool.tile([128, WQ], F32, tag="q", name="ps_qk")
                for cc in range(6):
                    nc.tensor.matmul(
                        ps[:, 0:WQ],
                        w_sb[:, cc, cp * 128:(cp + 1) * 128],
                        xT_sb[:, cc, g * WQ:(g + 1) * WQ],
                        start=(cc == 0),
                        stop=(cc == 5),
                    )
                dst = dest[:, j, g * WQ:(g + 1) * WQ]
                if on_act:
                    nc.scalar.copy(out=dst, in_=ps[:, 0:WQ])
                else:
                    nc.vector.tensor_copy(out=dst, in_=ps[:, 0:WQ])

            def emit_v(nt, pool, on_act):
                ps = pool.tile([128, WQ], F32, tag="q", name="ps_v")
                for cc in range(6):
                    nc.tensor.matmul(
                        ps[:, 0:384],
                        xT_sb[:, cc, nt * 128:(nt + 1) * 128],
                        w_sb[:, cc, 768:1152],
                        start=(cc == 0),
                        stop=(cc == 5),
                    )
                dst = v_ones[:, nt, :, 0:64]
                src = ps[:, 0:384].rearrange("p (h d) -> p h d", h=HG)
                if on_act:
                    nc.scalar.copy(out=dst, in_=src)
                else:
                    nc.vector.tensor_copy(out=dst, in_=src)

            def emit_norm_jp(qc, jp):
                q0 = qc * WQ
                r0 = qc * HG + 2 * jp
                pair = work2.tile([2, WQ], F32, tag="pair")
                nc.sync.dma_start(out=pair[:, :], in_=rscratch[r0:r0 + 2, :])
                nc.vector.reciprocal_approx_fast(out=pair[:, :], in_=pair[:, :])
                pairb = work2.tile([2, WQ], BF, tag="pairb")
                nc.vector.tensor_copy(out=pairb[:, :], in_=pair[:, :])
                nc.sync.dma_start(
                    out=rscratchb[r0:r0 + 2, :], in_=pairb[:, :]
                )
                bc = work2.tile([128, WQ], BF, tag="bcj")
                for half in range(2):
                    row = rscratchb[r0 + half:r0 + half + 1, :]
                    bsrc = bass.AP(
                        tensor=row.tensor,
                        offset=row.offset,
                        ap=[[0, 64], [1, WQ]],
                    )
                    nc.sync.dma_start(
                        out=bc[64 * half:64 * half + 64, :], in_=bsrc
                    )
                nc.vector.tensor_mul(
                    attn_sb[:, jp, q0:q0 + WQ],
                    attn_sb[:, jp, q0:q0 + WQ],
                    bc[:, :],
                )

            def emit_proj(nt, pool, on_act):
                for colh in range(2):
                    c0 = colh * 384
                    psy = pool.tile([128, WQ], F32, tag="q", name="ps_y")
                    for j in range(3):
                        nc.tensor.matmul(
                            psy[:, 0:384],
                            attn_sb[:, j, nt * 128:(nt + 1) * 128],
                            wp_sb[:, j, c0:c0 + 384],
                            start=(j == 0),
                            stop=(j == 2),
                        )
                    ysb = work2.tile([128, 384], BF, tag="ysb")
                    if on_act:
                        nc.scalar.copy(out=ysb[:, :], in_=psy[:, 0:384])
                    else:
                        nc.vector.tensor_copy(out=ysb[:, :], in_=psy[:, 0:384])
                    nc.gpsimd.dma_start(
                        out=y_d[nt * 128:(nt + 1) * 128, c0:c0 + 384],
                        in_=ysb[:, :],
                    )

            # ---------- prologue: qk(jp0) + v0,v1 on idle ACT ----------
            for cp in (0, 3):
                for g in range(4):
                    emit_qk_group(cp, g, ps_q, on_act=True)
            for nt in range(2):
                emit_v(nt, ps_q, on_act=True)

            # ---------- filler schedule: slot -> [callables] ----------
            # Constraints: v(nt) must be emitted at slot <= nt (consumer
            # O(nt) is emitted at slot nt+1); q/k plane groups before the
            # emit_s that reads them (jp1 from slot 16, jp2 from slot 32,
            # k-group g feeds k-tiles 4g..4g+3).
            fillers = {}

            def add_filler(slot, fn):
                fillers.setdefault(slot, []).append(fn)

            def f_v(nt):
                return lambda: emit_v(nt, ps_q, False)

            def f_qk(cp, g):
                return lambda: emit_qk_group(cp, g, ps_q, False)

            add_filler(0, f_v(2))
            for idx, nt in enumerate(range(3, NKT)):
                add_filler(1 + idx, f_v(nt))            # slots 1..13
            # jp1 planes needed from slot 64 (q g_qc at 64+16qc, k g_j at
            # 64+4j), jp2 from slot 128; spread groups to avoid overloading
            # the PE while v fillers drain.
            qk_sched = [
                (30, 4, 0), (34, 1, 0), (38, 4, 1), (44, 4, 2), (50, 4, 3),
                (56, 1, 1), (70, 1, 2), (86, 1, 3),
                (94, 5, 0), (98, 2, 0), (102, 5, 1), (108, 5, 2),
                (114, 5, 3), (120, 2, 1), (134, 2, 2), (150, 2, 3),
            ]
            for slot, cp, g in qk_sched:
                add_filler(slot, f_qk(cp, g))
            # proj(qc) after norm of (jp2, qc), which lands at slot
            # 128 + 16*(qc+1); spread the 4 n-tiles 3 slots apart
            for qc in range(NQC - 1):
                base = 128 + 16 * (qc + 1)
                for i in range(4):
                    add_filler(
                        base + 2 + 3 * i,
                        (lambda t: lambda: emit_proj(t, ps_q, False))(qc * 4 + i),
                    )

            # ---------- flat attention pipeline ----------
            # jp-outer: head-pair jp sweeps all 4 q-chunks before the next
            # pair starts, so jp1/jp2 qk planes are not needed until slots
            # 64/128 and the single-bank filler queue has ample slack.
            slots = [
                (qc, jp, kt)
                for jp in range(3)
                for qc in range(NQC)
                for kt in range(NKT)
            ]
            pT_pend = {}
            po_pend = {}

            def emit_s(qc, jp, kt):
                q0 = qc * WQ
                k0 = kt * 128
                psum_s = ps_s.tile([128, 2 * WQ], F32, tag="s")
                nc.tensor.matmul(
                    psum_s[:, 0:WQ],
                    kT_sb[0:64, jp, k0:k0 + 128],
                    qT_sb[0:64, jp, q0:q0 + WQ],
                    start=True,
                    stop=True,
                )
                nc.tensor.matmul(
                    psum_s[:, WQ:2 * WQ],
                    kT_sb[64:128, jp, k0:k0 + 128],
                    qT_sb[64:128, jp, q0:q0 + WQ],
                    start=True,
                    stop=True,
                )
                praw = work.tile([128, 2 * WQ], BF, tag="praw")
                nc.scalar.activation(
                    out=praw[:, :], in_=psum_s[:, :], func=EXP, scale=SCALE
                )
                pT = work.tile([128, 2 * WQ], BF, tag="pt")
                msl = m01_sb[:, kt, q0:q0 + WQ]
                msrc = bass.AP(
                    tensor=msl.tensor,
                    offset=msl.offset,
                    ap=[list(msl.ap[0]), [0, 2], [1, WQ]],
                )
                nc.vector.tensor_mul(
                    pT.rearrange("p (a b) -> p a b", a=2),
                    praw.rearrange("p (a b) -> p a b", a=2),
                    msrc,
                )
                pT_pend[(qc, jp, kt)] = pT

            def emit_o(qc, jp, kt):
                q0 = qc * WQ
                if kt == 0:
                    po_pend[(qc, jp)] = (
                        ps_o.tile([65, WQ], F32, tag="o", name="psum_oe"),
                        ps_o.tile([65, WQ], F32, tag="o", name="psum_oo"),
                    )
                psum_oe, psum_oo = po_pend[(qc, jp)]
                pT = pT_pend.pop((qc, jp, kt))
                nc.tensor.matmul(
                    psum_oe[:, :],
                    v_ones[:, kt, 2 * jp, :],
                    pT[:, 0:WQ],
                    start=(kt == 0),
                    stop=(kt == NKT - 1),
                )
                nc.tensor.matmul(
                    psum_oo[:, :],
                    v_ones[:, kt, 2 * jp + 1, :],
                    pT[:, WQ:2 * WQ],
                    start=(kt == 0),
                    stop=(kt == NKT - 1),
                )
                if kt == NKT - 1:
                    for half, po in ((0, psum_oe), (1, psum_oo)):
                        off = 64 * half
                        nc.vector.tensor_copy(
                            out=attn_sb[off:off + 64, jp, q0:q0 + WQ],
                            in_=po[0:64, :],
                        )
                        den = work2.tile([1, WQ], F32, tag="dn")
                        nc.vector.tensor_copy(out=den[:, :], in_=po[64:65, :])
                        row = qc * HG + 2 * jp + half
                        nc.sync.dma_start(
                            out=rscratch[row:row + 1, :], in_=den[:, :]
                        )
                    del po_pend[(qc, jp)]
                    emit_norm_jp(qc, jp)

            emit_s(*slots[0])
            for fn in fillers.get(0, ()):
                fn()
            for i in range(1, len(slots)):
                emit_s(*slots[i])
                emit_o(*slots[i - 1])
                for fn in fillers.get(i, ()):
                    fn()
            emit_o(*slots[-1])

            # ---------- tail: last q-chunk projection ----------
            for nt in range((NQC - 1) * 4, NQC * 4):
                emit_proj(nt, ps_q, on_act=True)


def _prep_inputs(x, mask, w_qkv, w_proj):
    """Build the 8 per-core input maps."""
    x = np.asarray(x, dtype=np.float32)
    mask = np.asarray(mask)
    w_qkv = np.asarray(w_qkv, dtype=np.float32)
    w_proj = np.asarray(w_proj, dtype=np.float32)

    m01T = np.ascontiguousarray((~mask).T.astype(np.float32)).astype(BF16)
    m01T = m01T.reshape(NKT, 128, N)

    w3 = w_qkv.reshape(C, 3, H, D)
    wp3 = w_proj.reshape(H, D, C)

    in_maps = []
    for core in range(NCORES):
        b, g = core // 2, core % 2
        hs = slice(g * HG, (g + 1) * HG)
        xT = np.ascontiguousarray(x[b].T).astype(BF16).reshape(6, 128, N)
        wq = w3[:, 0, hs, :].reshape(C, HG * D)
        wk = w3[:, 1, hs, :].reshape(C, HG * D)
        wv = w3[:, 2, hs, :].reshape(C, HG * D)
        wg = np.concatenate([wq, wk, wv], axis=1).astype(BF16)
        wg = np.ascontiguousarray(wg).reshape(6, 128, 1152)
        wp = np.ascontiguousarray(wp3[hs].reshape(3, 128, C)).astype(BF16)
        in_maps.append({"xT": xT, "wqkv": wg, "maskT": m01T, "wp": wp})
    return in_maps


_NC_CACHE = {}


def run_cores(in_maps, trace=False, **kw):
    if "nc" not in _NC_CACHE:
        _NC_CACHE["nc"] = _build_nc()
    nc = _NC_CACHE["nc"]
    return run_bass_kernel_spmd(
        nc, in_maps, core_ids=list(range(NCORES)), trace=trace, **kw
    )


def kernel(x, mask, w_qkv, w_proj, b_proj):
    in_maps = _prep_inputs(x, mask, w_qkv, w_proj)
    res = run_cores(in_maps)
    b_proj = np.asarray(b_proj, dtype=np.float32)
    out = np.empty((B, N, C), dtype=np.float32)
    for b in range(B):
        out[b] = (
            res.results[2 * b]["y"].astype(np.float32)
            + res.results[2 * b + 1]["y"].astype(np.float32)
            + b_proj
        )
    return out
